# revision 4
# baseline (speedup 1.0000x reference)
"""HakesIVF select_centers kernel v3: hybrid output materialization.

Per 8-tile group: tiles 2..7 use SWDGE indirect row-gathers (Pool engine,
~1.4us each, the machine's bottleneck); tiles 0..1 are materialized on the
PE instead, cutting Pool-engine work by 25%:

  V-op (DVE dual-stream): row max m over the 1024 scores (pair domain).
  H-op (DVE dual-stream): signed pair-hot  hot[i,k] = (max(a,b)==m) * (a>=b ? +1 : -1)
        accum ADD -> sigma = the +-1 sign of the winner's half.
  PE:   transpose hot (4x 128x128 fp16 blocks) -> psum; ACT copies to SBUF;
        4 K=128 matmuls vs host tables [Dif|S], Dif=(Ca-Cb)/2, S=(Ca+Cb)/2
        -> psum2 = [sigma*Dif[k*] | sigma*S[k*]].
  DVE:  out = sigma*psum2_S + psum2_Dif  = C[winner]   (scalar_tensor_tensor)

Scores, the pair-argmax scan, and the gather path are as kernel_v2.
"""

import numpy as np

N, NLIST, D = 262144, 1024, 64
NCORES = 8
NPC = N // NCORES
P = 128
NT = NPC // P
CHUNK = 32
GB = 8
NPE = 2                     # PE-materialized tiles per group (tiles 0..NPE-1)
K1 = D + 2
K2 = 2 * D

_cached = {}


def _register_ops():
    import numpy as np_

    from concourse import dve_ops
    from concourse.dve_spec import (
        AluOp, Bin, C0, MaxNeg, Spec, Src0, Src1, Zero, eq, lower, maxx, scan, select,
    )
    from concourse.dve_uop import DveOpSpec

    def _mk(name, spec, rd1):
        for op in dve_ops.OPS:
            if op.name == name:
                return op
        row = dve_ops._CUSTOM_DVE_ROW_BASE + len(dve_ops.OPS)
        assert row < 0x20
        uops_sha = {}
        for ver in ("v3", "v4"):
            compiled = DveOpSpec(name=name, opcode=row, uops=lower(spec, ver=ver), rd1_en=rd1)
            uops_sha[ver] = compiled.sha(ver)
        op = dve_ops.DveOp(name, spec, subdim=False, uops_sha=uops_sha)
        dve_ops.OPS.append(op)
        dve_ops.CUSTOM_DVE_SPECS[name] = spec
        dve_ops._SUB_OPCODE_FOR_NAME[name] = row
        return op

    def _ref_pair(in0, in1, s0, s1, imm2):
        a = np_.asarray(in0, dtype=np_.float32)
        b = np_.asarray(in1, dtype=np_.float32)
        m = np_.maximum(a, b)
        r = np_.maximum.accumulate(m, axis=-1)
        idx = np_.arange(m.shape[-1], dtype=np_.float32) * np_.float32(s0)
        enc = (idx + (a < b)).astype(np_.float32)
        miss = np_.float32(np_.finfo(np_.float32).min)
        body = np_.where(m == r, enc, miss).astype(np_.float32)
        return body, body.max(axis=-1, keepdims=True)

    m = maxx(Src0, Src1)
    r = scan(AluOp.MAX, m)
    idx2 = scan(AluOp.ADD, C0, init=Bin(AluOp.SUBTRACT, Zero, C0))
    enc = idx2 + Bin(AluOp.IS_LT, Src0, Src1)
    pair_op = _mk("PAIR_ARGMAX_ANT",
                  Spec(body=select(eq(m, r), enc, MaxNeg), accum=AluOp.MAX,
                       reference=_ref_pair), True)

    def _ref_v(in0, in1, s0, s1, imm2):
        a = np_.asarray(in0, dtype=np_.float32)
        b = np_.asarray(in1, dtype=np_.float32)
        body = np_.maximum(a, b).astype(np_.float32)
        return body, body.max(axis=-1, keepdims=True)

    v_op = _mk("PAIR_MAXV_ANT",
               Spec(body=maxx(Src0, Src1), accum=AluOp.MAX, reference=_ref_v), True)

    def _ref_h(in0, in1, s0, s1, imm2):
        a = np_.asarray(in0, dtype=np_.float32)
        b = np_.asarray(in1, dtype=np_.float32)
        m = np_.maximum(a, b)
        s0a = np_.asarray(s0, dtype=np_.float32)
        while s0a.ndim < m.ndim:
            s0a = s0a[..., None]
        sgn = np_.where(a >= b, np_.float32(1.0), np_.float32(-1.0))
        body = ((m == s0a).astype(np_.float32) * sgn).astype(np_.float32)
        return body, body.sum(axis=-1, keepdims=True)

    hbody = eq(maxx(Src0, Src1), C0) * (
        Bin(AluOp.IS_GE, Src0, Src1) - Bin(AluOp.IS_LT, Src0, Src1)
    )
    h_op = _mk("PAIR_HOT_ANT",
               Spec(body=hbody, accum=AluOp.ADD, reference=_ref_h), True)
    return pair_op, v_op, h_op



def _indirect_gather_q(nc, mybir, bass, out, in_, offset_ap, queue_num):
    """indirect_dma_start clone with SWDGE queue selection (round-robin)."""
    b = nc.gpsimd
    assert in_.space == bass.MemorySpace.DRAM and out.space == bass.MemorySpace.SBUF
    assert isinstance(in_.offset, int) and in_.offset == 0
    out_ap = b.lower_ap_dma(out, for_indirect_dma=True)
    in_ap = b.lower_ap_dma(in_, for_indirect_dma=True)
    assert len(in_ap) == 1 and len(out_ap) == 1
    off = b.lower_ap_dma(offset_ap)
    assert len(off) == 1
    in_ap.append(off[0])
    ap_shape = in_.shape
    coef = 1
    for i in range(1, len(ap_shape)):
        coef *= ap_shape[i]
    in_ap[0].dynamic_ap_info = mybir.DynamicAccessPatternInfo(
        c=0, actual_ap=out.ap,
        indirect_dim_max_index=ap_shape[0],
        offset_expr=[mybir.DynamicAccessPatternOffsetExpr(
            coef=coef,
            aff_expr=mybir.DynamicAccessPatternOffsetExprAffExpr(
                kind="IndirectArgId", arg_id=1))])
    qname = "qPoolDynamic" if queue_num == 0 else f"qPoolDynamic{queue_num}"
    return b.add_instruction(mybir.InstDMACopy(
        name=b.bass.get_next_instruction_name(),
        queue=qname, mode="Copy", ins=in_ap, outs=out_ap,
        oob_is_err=True, cce_op=mybir.AluOpType.bypass))


def build_nc(npc=NPC, nlist=NLIST, d=D, chunk=CHUNK, n_queues=4):
    from contextlib import ExitStack

    import concourse.tile as tile
    from concourse import bacc, bass, mybir

    k1, k2 = d + 2, 2 * d
    nt = npc // P
    half = nlist // 2
    assert nt % chunk == 0 and chunk % GB == 0
    fp32 = mybir.dt.float32
    fp16 = mybir.dt.float16
    u32 = mybir.dt.uint32

    pair_op, v_op, h_op = _register_ops()

    nc = bacc.Bacc("TRN2", target_bir_lowering=False, debug=False,
                   num_devices=NCORES, num_swdge_queues=n_queues)
    vt1 = nc.dram_tensor("vt1", [k1, npc], fp16, kind="ExternalInput").ap()
    vt2 = nc.dram_tensor("vt2", [k2, npc], fp16, kind="ExternalInput").ap()
    ct1 = nc.dram_tensor("ct1", [k1, nlist], fp16, kind="ExternalInput").ap()
    ct2 = nc.dram_tensor("ct2", [k2, nlist], fp16, kind="ExternalInput").ap()
    ctab2 = nc.dram_tensor("ctab2", [nlist, d], fp32, kind="ExternalInput").ap()
    dstab = nc.dram_tensor("dstab", [P, 4 * P], fp16, kind="ExternalInput").ap()
    ident = nc.dram_tensor("ident", [P, P], fp16, kind="ExternalInput").ap()
    out = nc.dram_tensor("out", [npc, d], fp32, kind="ExternalOutput").ap()

    with tile.TileContext(nc) as tc, ExitStack() as ctx:
        const_pool = ctx.enter_context(tc.tile_pool(name="const", bufs=1))
        vchunk_pool = ctx.enter_context(tc.tile_pool(name="vchunk", bufs=3))
        psA_pool = ctx.enter_context(tc.tile_pool(name="psA", bufs=4, space="PSUM"))
        psB_pool = ctx.enter_context(tc.tile_pool(name="psB", bufs=2, space="PSUM"))
        psT_pool = ctx.enter_context(tc.tile_pool(name="psT", bufs=1, space="PSUM"))
        ps2_pool = ctx.enter_context(tc.tile_pool(name="ps2", bufs=1, space="PSUM"))
        sbb_pool = ctx.enter_context(tc.tile_pool(name="sbb", bufs=6))
        scr_pool = ctx.enter_context(tc.tile_pool(name="scr", bufs=4))
        hot_pool = ctx.enter_context(tc.tile_pool(name="hot", bufs=4))
        acc_pool = ctx.enter_context(tc.tile_pool(name="acc", bufs=6))
        gout_pool = ctx.enter_context(tc.tile_pool(name="gout", bufs=6))

        ct1_sb = const_pool.tile([k1, nlist], fp16)
        nc.sync.dma_start(ct1_sb[:], ct1[:])
        ct2_sb = const_pool.tile([k2, nlist], fp16)
        nc.sync.dma_start(ct2_sb[:], ct2[:])
        ds_sb = const_pool.tile([P, 4 * P], fp16)
        id_sb = const_pool.tile([P, P], fp16)

        for c in range(nt // chunk):
            vch1 = vchunk_pool.tile([k1, chunk * P], fp16, tag="vch1")
            vch2 = vchunk_pool.tile([k2, chunk * P], fp16, tag="vch2")
            base = c * chunk * P
            # chunk 0: fine-grained strips so tile 0 compute starts ASAP
            nparts = 8 if c == 0 else 2
            sp = chunk * P // nparts
            for s in range(nparts):
                nc.sync.dma_start(vch1[:, s * sp : (s + 1) * sp],
                                  vt1[:, base + s * sp : base + (s + 1) * sp])
                nc.sync.dma_start(vch2[:, s * sp : (s + 1) * sp],
                                  vt2[:, base + s * sp : base + (s + 1) * sp])
                if c == 0 and s == 0:
                    nc.sync.dma_start(ds_sb[:], dstab[:])
                    nc.sync.dma_start(id_sb[:], ident[:])

            for g0 in range(0, chunk, GB):
                # last group of each chunk materializes 3 tiles on the PE
                # (f=17/64: equalizes the DVE and gather stream end-times)
                npe = 3 if (g0 // GB) % (chunk // GB) == (chunk // GB) - 1 else NPE
                accf = acc_pool.tile([P, GB], fp32, tag="accf")
                mval = acc_pool.tile([P, 3], fp32, tag="mval")
                sgm = acc_pool.tile([P, 3], fp32, tag="sgm")
                hots = []
                for i in range(GB):
                    g = g0 + i
                    w1 = vch1[:, g * P : (g + 1) * P]
                    w2 = vch2[:, g * P : (g + 1) * P]
                    psA = psA_pool.tile([P, half], fp32)
                    psB = psB_pool.tile([P, half], fp32)
                    nc.tensor.matmul(psB[:], lhsT=w1, rhs=ct1_sb[:, half:],
                                     start=True, stop=False)
                    nc.tensor.matmul(psA[:], lhsT=w1, rhs=ct1_sb[:, :half],
                                     start=True, stop=False)
                    nc.tensor.matmul(psB[:], lhsT=w2, rhs=ct2_sb[:, half:],
                                     start=False, stop=True)
                    nc.tensor.matmul(psA[:], lhsT=w2, rhs=ct2_sb[:, :half],
                                     start=False, stop=True)
                    sbb = sbb_pool.tile([P, half], fp32, tag="sbb")
                    nc.scalar.copy(sbb[:], psB[:])
                    scr = scr_pool.tile([P, half], fp32, tag="scr")
                    if i >= npe:
                        nc.vector._custom_dve(
                            pair_op, out=scr[:], in0=psA[:], in1=sbb[:],
                            s0=2.0, accum_out=accf[:, i : i + 1])
                    else:
                        nc.vector._custom_dve(
                            v_op, out=scr[:], in0=psA[:], in1=sbb[:],
                            accum_out=mval[:, i : i + 1])
                        hot = hot_pool.tile([P, half], fp16, tag="hot")
                        nc.vector._custom_dve(
                            h_op, out=hot[:], in0=psA[:], in1=sbb[:],
                            s0=mval[:, i : i + 1], accum_out=sgm[:, i : i + 1])
                        hots.append(hot)

                idxu = acc_pool.tile([P, GB - npe], u32, tag="idxu")
                nc.vector.tensor_copy(idxu[:], accf[:, npe:])
                gout = gout_pool.tile([P, GB * d], fp32)
                for i in range(npe, GB):
                    _indirect_gather_q(
                        nc, mybir, bass,
                        gout[:, i * d : (i + 1) * d], ctab2[:],
                        idxu[:, i - npe : i - npe + 1],
                        (c * chunk + g0 + i) % 4)

                # PE materialization, in batches of <=2 tiles so psT/ps2 stay
                # one bank each (3rd tile reuses them via pool serialization)
                for b0 in range(0, npe, 2):
                    bts = list(range(b0, min(b0 + 2, npe)))
                    nb = len(bts)
                    psT = psT_pool.tile([P, nb * half], fp16)
                    ps2 = ps2_pool.tile([P, nb * 2 * d], fp32)
                    for j, i in enumerate(bts):
                        hot = hots[i]
                        for b in range(4):
                            nc.tensor.matmul(
                                psT[:, j * half + b * P : j * half + (b + 1) * P],
                                lhsT=hot[:, b * P : (b + 1) * P],
                                rhs=id_sb[:], is_transpose=True)
                    hotT = hot_pool.tile([P, nb * half], fp16, tag="hotT")
                    nc.scalar.copy(hotT[:], psT[:])
                    for j, i in enumerate(bts):
                        p2 = ps2[:, j * 2 * d : (j + 1) * 2 * d]
                        for b in range(4):
                            nc.tensor.matmul(
                                p2, lhsT=hotT[:, j * half + b * P : j * half + (b + 1) * P],
                                rhs=ds_sb[:, b * P : (b + 1) * P],
                                start=(b == 0), stop=(b == 3))
                    s2 = hot_pool.tile([P, nb * 2 * d], fp32, tag="s2")
                    nc.scalar.copy(s2[:], ps2[:])
                    for j, i in enumerate(bts):
                        sl = s2[:, j * 2 * d : (j + 1) * 2 * d]
                        nc.vector.scalar_tensor_tensor(
                            out=gout[:, i * d : (i + 1) * d],
                            in0=sl[:, d:], scalar=sgm[:, i : i + 1], in1=sl[:, :d],
                            op0=mybir.AluOpType.mult, op1=mybir.AluOpType.add)

                r0 = (c * chunk + g0) * P
                dst = out[r0 : r0 + GB * P, :].rearrange("(g p) d -> p g d", p=P)
                nc.sync.dma_start(dst, gout[:].rearrange("p (g d) -> p g d", d=d))

    nc.compile()
    return nc


def _split16(a):
    hi = a.astype(np.float16)
    lo = (a - hi.astype(np.float32)).astype(np.float16)
    return hi, lo


def _prep_inputs(vecs, centroids):
    vecs = np.ascontiguousarray(np.asarray(vecs), dtype=np.float32)
    cents = np.ascontiguousarray(np.asarray(centroids), dtype=np.float32)
    csq = np.sum(cents * cents, axis=1, dtype=np.float32)
    b1, b2 = _split16(-0.5 * csq)
    g1, g2 = _split16(cents)

    ct1 = np.empty((K1, NLIST), dtype=np.float16)
    ct1[:D] = g1.T
    ct1[D] = b1
    ct1[D + 1] = b2
    ct2 = np.empty((K2, NLIST), dtype=np.float16)
    ct2[:D] = g2.T
    ct2[D:] = g1.T

    ctab2 = np.empty((NLIST, D), dtype=np.float32)
    ctab2[0::2] = cents[: NLIST // 2]
    ctab2[1::2] = cents[NLIST // 2 :]

    # [Dif | S] table: pair k = (cents[k], cents[512+k]), partitioned as
    # dstab[q, b*128 : b*128+128] = [Dif[128b+q] (64) | S[128b+q] (64)]
    ca = cents[: NLIST // 2]
    cb = cents[NLIST // 2 :]
    dif = (ca - cb) / 2.0
    s = (ca + cb) / 2.0
    dstab = np.empty((P, 4 * P), dtype=np.float16)
    for b in range(4):
        for q in range(P):
            dstab[q, b * P : b * P + D] = dif[b * P + q]
            dstab[q, b * P + D : b * P + 2 * D] = s[b * P + q]

    ident = np.eye(P, dtype=np.float16)

    in_maps = []
    for c in range(NCORES):
        sl = vecs[c * NPC : (c + 1) * NPC]
        h1, h2 = _split16(sl)
        vt1 = np.empty((K1, NPC), dtype=np.float16)
        vt1[:D] = h1.T
        vt1[D:] = 1.0
        vt2 = np.empty((K2, NPC), dtype=np.float16)
        vt2[:D] = h1.T
        vt2[D:] = h2.T
        in_maps.append({"vt1": vt1, "vt2": vt2, "ct1": ct1, "ct2": ct2,
                        "ctab2": ctab2, "dstab": dstab, "ident": ident})
    return in_maps


def kernel(vecs, centroids):
    from concourse.bass_utils import run_bass_kernel_spmd

    if "nc" not in _cached:
        _cached["nc"] = build_nc()
    nc = _cached["nc"]

    in_maps = _prep_inputs(vecs, centroids)
    res = run_bass_kernel_spmd(nc, in_maps, core_ids=list(range(NCORES)))
    outs = [res.results[c]["out"] for c in range(NCORES)]
    return np.concatenate(outs, axis=0)
